# revision 86
# baseline (speedup 1.0000x reference)
"""Trainium2 Bass kernel for the DWT block (dense_cnn), 8-core SPMD.

Sharding: H-dim spatial across 8 NeuronCores, halos baked into per-core inputs
(no halo exchange). Border semantics (conv zero-pad, reflect blur, DWT odd-row,
channel-attention row ownership) are data-ized into per-core mask / tap-weight
inputs so every core runs ONE uniform program. The channel-attention global
mean/max is the only collective (512B AllGather of partial stats).

Perf structure (vs. the original baseline):
 - constants packed into 4 DMAs (was 22)
 - DWT/blur pipeline in f16 (2x DVE) with DMA column-parity de-interleave
 - vertical gaussian blur as one banded-matmul pair (was 5 DVE MAC passes)
 - 6x6 cat conv as 5 banded matmuls with K=126 (was 83 matmuls with K=6)
 - tmp12h evacuated per-16-rows so DWT chunks overlap the conv1/2/3 phase
 - DMA routing split across SP/Act HWDGE queues and the Pool SWDGE queue
 - f16 output stores (host casts back to f32)
"""
import sys
for _p in ('/opt/trn_rl_repo',):
    if _p not in sys.path:
        sys.path.insert(0, _p)
import numpy as np
import concourse.bass as bass
import concourse.bacc as bacc
import concourse.tile as tile
from concourse import mybir
from concourse.alu_op_type import AluOpType

F32 = mybir.dt.float32
F16 = mybir.dt.float16
RELU = mybir.ActivationFunctionType.Relu
SIG = mybir.ActivationFunctionType.Sigmoid

# ---------------- geometry (uniform across cores) ----------------
NCORE = 8
H = W = 511
NOUT = 64            # out rows per core (core 7: last row garbage)
NF = 68              # f12 rows [r0-2, r0+66)
NH = 73              # hf/hb/vb rows [r0-2, r0+71)
NHP = 77             # hf_pre rows [r0-4, r0+73)
NQ = 39              # hi rows [32i-2, 32i+37)
NZ = 39              # cHVD rows [32i-3, 32i+36)
NT = 78              # tmp12/x rows [64i-6, 64i+72)
G4 = (NT + 3) // 4   # 20 conv1 4-row groups (80 rows, 2 garbage)
NY16 = (NT + 15) // 16  # 5
NHB = 128            # hb padded partition count
CATY = 3             # cat-conv blocks of 32 output rows
CATKR = 37           # cat lhsT contraction rows (krow 0..36)
FRROWS = 70          # f12r replicated row window (c9a reads rows <= 69)

# const pack column layouts
CP16_L1A, CP16_L1B, CP16_L2, CP16_L3 = 0, 32, 64, 96
CP16_L9A, CP16_L9B, CP16_LCAT = 128, 256, 768
CP16_TOT = 768 + 3 * 6 * 32
CP32_B1, CP32_B2, CP32_B9B, CP32_CBHI, CP32_B3 = 0, 1, 2, 3, 7
CP32_TOT = 8
KP16_GV, KP16_RBV, KP16_CB = 0, 73, 589
KP16_TOT = 593
KP32_OM, KP32_OMN, KP32_RMV, KP32_SMV, KP32_SAV, KP32_TMASK = 0, 1, 2, 3, 4, 5
KP32_BAP, KP32_FMY, KP32_BCY = 6, 23, 26
KP32_TOT = 29


def core_rows(i):
    r0 = 64 * i
    return dict(r0=r0, t0=r0 - 6, f0=r0 - 2, h0=r0 - 2, p0=r0 - 4,
                q0=32 * i - 2, z0=32 * i - 3, a0=r0 - 1)


def gaussk():
    r = np.arange(5, dtype=np.float64) - 2.0
    k = np.exp(-(r ** 2) / (2.0 * 0.25))
    return (k / k.sum()).astype(np.float64)


# ---------------- host-side input prep ----------------
def prep_shared(inp):
    """Host-prepped tensors identical on all cores."""
    w1 = np.asarray(inp['w_x1']).reshape(16, 204)
    w2 = np.asarray(inp['w_x2']).reshape(8, 16)
    w3 = np.asarray(inp['w_x3']).reshape(1, 8)
    whi = np.asarray(inp['w_hi']).reshape(4, 3) * 0.5   # fold DWT 1/2 scale
    wcat = np.asarray(inp['w_cat'])[0, 0]               # (6,6) ch0; ch1 dead
    w9a = np.asarray(inp['w_c9a']).reshape(32, 3, 3)
    w9b = np.asarray(inp['w_c9b'])                      # (64,32,3,3)

    p16 = np.zeros((128, CP16_TOT), np.float16)
    p16[:, CP16_L1A:CP16_L1A + 16] = w1[:, :128].T
    p16[:76, CP16_L1B:CP16_L1B + 16] = w1[:, 128:].T
    for j in range(4):
        for c in range(16):
            for o in range(8):
                p16[32 * j + c, CP16_L2 + 8 * j + o] = w2[o, c]
    for gq in range(4):
        for j in range(4):
            for o in range(8):
                p16[32 * gq + 8 * j + o, CP16_L3 + 4 * gq + j] = w3[0, o]
    # c9a lhsT [18, 128]: partition 3*krow+kx -> out 32j+c with ky=krow-j
    for krow in range(6):
        for kx in range(3):
            for j in range(4):
                ky = krow - j
                if 0 <= ky <= 2:
                    for c in range(32):
                        p16[3 * krow + kx, CP16_L9A + 32 * j + c] = w9a[c, ky, kx]
    # c9b lhsT per dy chunk [96, 4*128]
    for dy in range(4):
        for j2 in range(2):
            ky = dy - j2
            if 0 <= ky <= 2:
                for dx in range(3):
                    for c in range(32):
                        for co in range(64):
                            p16[32 * dx + c, CP16_L9B + 128 * dy + 64 * j2 + co] = \
                                w9b[co, c, ky, dx]
    # cat lhsT per kx [37, 32]: partition krow -> out row m with ky=krow-m.
    # One copy per 32-row output block Y at partition base 32Y (matmul needs
    # lhsT/rhs co-based); own column window per Y since 37-row copies overlap.
    for Y in range(3):
        for kx in range(6):
            for krow in range(CATKR):
                for m in range(32):
                    ky = krow - m
                    if 0 <= ky <= 5:
                        p16[32 * Y + krow, CP16_LCAT + 192 * Y + 32 * kx + m] = wcat[ky, kx]

    p32 = np.zeros((128, CP32_TOT), np.float32)
    for j in range(4):
        p32[32 * j:32 * j + 16, CP32_B1] = np.asarray(inp['b_x1'])
    for gq in range(4):
        for j in range(4):
            p32[32 * gq + 8 * j:32 * gq + 8 * j + 8, CP32_B2] = np.asarray(inp['b_x2'])
    p32[:64, CP32_B9B] = np.asarray(inp['b_c9b'])
    p32[64:, CP32_B9B] = np.asarray(inp['b_c9b'])
    for ch in range(4):
        p32[:NZ, CP32_CBHI + ch] = float(np.asarray(inp['b_hi']).reshape(4)[ch])
    p32[:, CP32_B3] = float(np.asarray(inp['b_x3']).reshape(()))

    d = {'cpk16': p16, 'cpk32': p32}
    sc = {}
    sc['b3'] = float(np.asarray(inp['b_x3']).reshape(()))
    sc['bhi'] = [float(b) for b in np.asarray(inp['b_hi']).reshape(4)]
    sc['whi'] = whi.astype(np.float64)
    sc['g'] = gaussk()
    sc['bcat'] = float(np.asarray(inp['b_cat']).reshape(()))
    sc['b9a'] = np.asarray(inp['b_c9a']).reshape(32).astype(np.float64)
    sc['ca_w1'] = float(np.asarray(inp['ca1_w1']).reshape(()))
    sc['ca_w2'] = float(np.asarray(inp['ca1_w2']).reshape(()))
    return d, sc


def prep_core(inp, i, sc):
    """Per-core data inputs."""
    g = sc['g']
    cr = core_rows(i)
    x = np.asarray(inp['x'])[0]          # [204, 511, 511]
    rows = np.clip(np.arange(cr['t0'], cr['t0'] + NT), 0, H - 1)
    xs = x[:, rows, :]                   # [204, NT, 511] edge-clamped
    d = {}
    d['xa'] = np.ascontiguousarray(xs[:128]).astype(np.float16)
    d['xb'] = np.ascontiguousarray(xs[128:]).astype(np.float16)

    k32 = np.zeros((128, KP32_TOT), np.float32)
    # DWT odd-row mask (+/- variants)
    for lz in range(NZ):
        ok = 0 <= 2 * (cr['z0'] + lz) + 1 <= 510
        k32[lz, KP32_OM] = 1.0 if ok else 0.0
        k32[lz, KP32_OMN] = -1.0 if ok else 0.0
    # hf_pre row mask + border rows
    rb = np.asarray([max(0.0, b) for b in sc['bhi']], np.float32)  # relu(b_hi)
    rowborder = np.zeros((NHP, 516), np.float32)
    rowmask = np.ones((NHP, 1), np.float32)
    for lp in range(NHP):
        p = cr['p0'] + lp
        if p in (0, 1, 514, 515):
            r = p % 2
            rowmask[lp] = 0.0
            rowborder[lp, 0::2] = rb[2 * r + 0]
            rowborder[lp, 1::2] = rb[2 * r + 1]
        elif not (0 <= p <= 515):
            rowmask[lp] = 0.0
    k32[:NHP, KP32_RMV] = rowmask[:, 0]
    # border cols 0,1,514,515
    cb = np.zeros((NHP, 4), np.float32)
    for lp in range(NHP):
        p = cr['p0'] + lp
        r = abs(p) % 2
        if 0 <= p <= 515 and p not in (0, 1, 514, 515):
            cb[lp] = [rb[2 * r], rb[2 * r + 1], rb[2 * r], rb[2 * r + 1]]
        else:
            cb[lp] = rowborder[lp, [0, 1, 514, 515]] if 0 <= p <= 515 else 0.0
    # stats masks over hb rows (owned global hf rows)
    ow0, ow1 = (516 * i) // 8, (516 * (i + 1)) // 8
    for lv in range(NH):
        y = cr['h0'] + lv
        own = ow0 <= y < ow1
        k32[lv, KP32_SMV] = 1.0 if own else 0.0
        k32[lv, KP32_SAV] = 0.0 if own else -60000.0
    # f12 row validity (tmp12 side)
    for f in range(NF):
        k32[f, KP32_TMASK] = 1.0 if 0 <= cr['f0'] + f <= 510 else 0.0
    # c9a bias' trick: b9a for valid rows, big-negative for garbage rows
    for Y in range(17):
        for j in range(4):
            ok = 0 <= cr['a0'] + 4 * Y + j <= 510
            for c in range(32):
                k32[32 * j + c, KP32_BAP + Y] = sc['b9a'][c] if ok else -30000.0
    # cat evac masks: fm (0/1) and bcat*fm per (m, Y32)
    for Y in range(CATY):
        for m in range(32):
            f = 32 * Y + m
            ok = f < NF and 0 <= cr['f0'] + f <= 510
            k32[m, KP32_FMY + Y] = 1.0 if ok else 0.0
            k32[m, KP32_BCY + Y] = sc['bcat'] if ok else 0.0
    d['kpk32'] = k32

    k16 = np.zeros((128, KP16_TOT), np.float16)
    # vertical blur banded lhsT with reflect folded: gv[p=lv+off, lv]
    for lv in range(NH):
        y = cr['h0'] + lv
        if 0 <= y <= 515:
            acc = {}
            for a in range(5):
                yy = y + a - 2
                if yy < 0:
                    yy = -yy
                if yy > 515:
                    yy = 1030 - yy
                off = yy - y + 2
                acc[off] = acc.get(off, 0.0) + g[a]
            for off, val in acc.items():
                assert 0 <= off <= 4
                k16[lv + off, KP16_GV + lv] = val
    # rowborder in parity-split layout [77, 516] = (s,258)
    rbps = np.concatenate([rowborder[:, 0::2], rowborder[:, 1::2]], axis=1)
    k16[:NHP, KP16_RBV:KP16_RBV + 516] = rbps
    k16[:NHP, KP16_CB:KP16_CB + 4] = cb
    d['kpk16'] = k16
    return d


# ---------------- device program ----------------
def build_kernel(sc, dbg=False, sim=False, upto='full'):
    nc = bacc.Bacc("TRN2", target_bir_lowering=False, debug=False, num_devices=NCORE)
    D = {}

    def din(name, shape, dt):
        D[name] = nc.dram_tensor(name, list(shape), dt, kind="ExternalInput")
        return D[name]

    xa_d = din('xa', [128, NT, 511], F16)
    xb_d = din('xb', [76, NT, 511], F16)
    cpk16_d = din('cpk16', [128, CP16_TOT], F16)
    cpk32_d = din('cpk32', [128, CP32_TOT], F32)
    kpk32_d = din('kpk32', [128, KP32_TOT], F32)
    kpk16_d = din('kpk16', [128, KP16_TOT], F16)
    out_d = nc.dram_tensor("out", [128, 32, 511], F16, kind="ExternalOutput")
    if dbg:
        dtmp_d = nc.dram_tensor("dbg_tmp", [128, 2, 512], F32, kind="ExternalOutput")
        dhb_d = nc.dram_tensor("dbg_hb", [NH, 516], F32, kind="ExternalOutput")
        dst_d = nc.dram_tensor("dbg_stats", [1, 8], F32, kind="ExternalOutput")
        df12_d = nc.dram_tensor("dbg_f12", [96, 513], F32, kind="ExternalOutput")
        dh0_d = nc.dram_tensor("dbg_h0", [128, 17, 511], F32, kind="ExternalOutput")

    g = sc['g']
    whi = sc['whi']
    b3 = sc['b3']
    with tile.TileContext(nc) as tc:
        def _emit():
            import contextlib as _cl
            ctx = _cl.ExitStack()
            with ctx:
                consts = ctx.enter_context(tc.tile_pool(name="consts", bufs=1))
                cpk16 = consts.tile([128, CP16_TOT], F16, name="cpk16")
                cpk32 = consts.tile([128, CP32_TOT], F32, name="cpk32")
                kpk32 = consts.tile([128, KP32_TOT], F32, name="kpk32")
                kpk16 = consts.tile([128, KP16_TOT], F16, name="kpk16")
                nc.sync.dma_start(out=cpk16[:], in_=cpk16_d[:])
                nc.sync.dma_start(out=cpk32[:], in_=cpk32_d[:])
                nc.scalar.dma_start(out=kpk32[:], in_=kpk32_d[:])
                nc.scalar.dma_start(out=kpk16[:], in_=kpk16_d[:])
                l1a = cpk16[0:128, CP16_L1A:CP16_L1A + 32]
                l1b = cpk16[0:76, CP16_L1B:CP16_L1B + 32]
                l2 = cpk16[0:128, CP16_L2:CP16_L2 + 32]
                l3 = cpk16[0:128, CP16_L3:CP16_L3 + 32]
                l9a = cpk16[0:18, CP16_L9A:CP16_L9A + 128]
                b1v = cpk32[0:128, CP32_B1:CP32_B1 + 1]
                b2v = cpk32[0:128, CP32_B2:CP32_B2 + 1]
                b9bv = cpk32[0:128, CP32_B9B:CP32_B9B + 1]

                mid = ctx.enter_context(tc.tile_pool(name="mid", bufs=1))
                dram = ctx.enter_context(tc.tile_pool(name="dram", bufs=1, space="DRAM"))
                tmp12h = mid.tile([128, 2, 512], F16, name="tmp12h")
                pairs = mid.tile([NZ, 2, 512], F16, name="pairs")
                PQ = mid.tile([NZ, 2, 256], F16, name="PQ")
                MN = mid.tile([NZ, 2, 256], F16, name="MN")
                cHt = mid.tile([NZ, 256], F16, name="cHt")
                cVt = mid.tile([NZ, 256], F16, name="cVt")
                cDt = mid.tile([NZ, 256], F16, name="cDt")
                his_all = mid.tile([NZ, 4, 256], F16, name="his_all")
                hfp = mid.tile([NHP, 2, 258], F16, name="hfp")
                vb = mid.tile([NH, 2, 258], F16, name="vb")
                hbe = mid.tile([NH, 258], F16, name="hbe")
                hbo = mid.tile([NH, 258], F16, name="hbo")
                hb = mid.tile([NHB, 516], F16, name="hb")
                hbsh = mid.tile([NHB, 516], F16, name="hbsh")
                f12dense = mid.tile([96, 513], F16, name="f12dense")

                # early memsets (DVE; all tiny; 32-aligned partition starts)
                nc.vector.memset(tmp12h[:, :, 511:512], 0.0)
                nc.vector.memset(hb[64:NHB, :], 0.0)
                nc.vector.memset(f12dense[:, 0:1], 0.0)
                nc.vector.memset(f12dense[:, 512:513], 0.0)
                # hfp border columns are constants: land them at t=0, off the
                # blur critical path (the border STT later skips cols 0/257)
                nc.gpsimd.dma_start(out=hfp[:, :, 0:1],
                                    in_=kpk16[0:NHP, KP16_CB:KP16_CB + 2])
                nc.gpsimd.dma_start(out=hfp[:, :, 257:258],
                                    in_=kpk16[0:NHP, KP16_CB + 2:KP16_CB + 4])
                # preload the act-func table set that covers Relu AND Sigmoid
                # so the mid-kernel sigmoid doesn't force a 1.3us table swap
                sigwarm = mid.tile([1, 1], F32, name="sigwarm")
                nc.scalar.activation(sigwarm[:], cpk32[0:1, 0:1], SIG, bias=0.0, scale=1.0)
                ccin = mid.tile([1, 128], F32, name="ccin")
                nc.vector.memset(ccin[:], 0.0)

                omp = kpk32[0:NZ, KP32_OM:KP32_OM + 1]
                omn = kpk32[0:NZ, KP32_OMN:KP32_OMN + 1]
                rmv = kpk32[0:NHP, KP32_RMV:KP32_RMV + 1]
                smv = kpk32[0:NH, KP32_SMV:KP32_SMV + 1]
                sav = kpk32[0:NH, KP32_SAV:KP32_SAV + 1]
                tmaskc = kpk32[0:NF, KP32_TMASK:KP32_TMASK + 1]
                gv = kpk16[0:NHP, KP16_GV:KP16_GV + NH]
                rbvv = kpk16[0:NHP, KP16_RBV:KP16_RBV + 516]

                # ---- DWT chunk emission (interleaved with conv groups) ----
                # za must be 32-aligned (engine partition-start constraint);
                # chunks lists which 8-z pair-DMA chunks to load this round.
                def emit_dwt_group(za, zb, chunks, scat_lo=None, tail=False):
                    # tail groups route DMAs via the (by then idle) HWDGE
                    # queues; early groups stay on Pool/SWDGE so they can't
                    # head-of-line block the x loads on SP
                    dma_eng = nc.sync if tail else nc.gpsimd
                    pr_ap = pairs[:]
                    th_ap = tmp12h[:]
                    for a in chunks:
                        t = a // 4
                        pb = 32 * (a % 4)
                        lt0 = 16 * a
                        ne = min(8, (NT - lt0 + 1) // 2)
                        no = min(8, (NT - lt0) // 2)
                        src_e = bass.AP(tensor=th_ap.tensor,
                                        offset=th_ap.offset + pb * 1024 + t * 512,
                                        ap=[[2048, ne], [1, 512]])
                        dst_e = bass.AP(tensor=pr_ap.tensor,
                                        offset=pr_ap.offset + (8 * a) * 1024,
                                        ap=[[1024, ne], [1, 512]])
                        dma_eng.dma_start(out=dst_e, in_=src_e)
                        src_o = bass.AP(tensor=th_ap.tensor,
                                        offset=th_ap.offset + (pb + 1) * 1024 + t * 512,
                                        ap=[[2048, no], [1, 512]])
                        dst_o = bass.AP(tensor=pr_ap.tensor,
                                        offset=pr_ap.offset + (8 * a) * 1024 + 512,
                                        ap=[[1024, no], [1, 512]])
                        dma_eng.dma_start(out=dst_o, in_=src_o)
                    sl = slice(za, zb)
                    a_ = pairs[sl, :, 0:512:2]
                    b_ = pairs[sl, :, 1:512:2]
                    nc.vector.tensor_tensor(PQ[sl, :, :], a_, b_, AluOpType.add)
                    nc.vector.tensor_tensor(MN[sl, :, :], a_, b_, AluOpType.subtract)
                    # cH = P - Q*om ; cV = M + N2*om ; cD = M - N2*om
                    nc.vector.scalar_tensor_tensor(cHt[sl, :], PQ[sl, 1, :], omn[sl, :],
                                                   PQ[sl, 0, :], AluOpType.mult, AluOpType.add)
                    nc.vector.scalar_tensor_tensor(cVt[sl, :], MN[sl, 1, :], omp[sl, :],
                                                   MN[sl, 0, :], AluOpType.mult, AluOpType.add)
                    nc.vector.scalar_tensor_tensor(cDt[sl, :], MN[sl, 1, :], omn[sl, :],
                                                   MN[sl, 0, :], AluOpType.mult, AluOpType.add)
                    for ch in range(4):
                        eng = nc.vector
                        hi_t = his_all[sl, ch, :]
                        bcol = cpk32[sl, CP32_CBHI + ch:CP32_CBHI + ch + 1]
                        eng.tensor_scalar_mul(hi_t, cHt[sl, :], float(whi[ch, 0]))
                        eng.scalar_tensor_tensor(hi_t, cVt[sl, :], float(whi[ch, 1]),
                                                 hi_t, AluOpType.mult, AluOpType.add)
                        eng.scalar_tensor_tensor(hi_t, cDt[sl, :], float(whi[ch, 2]),
                                                 hi_t, AluOpType.mult, AluOpType.add)
                        eng.tensor_scalar(hi_t, hi_t, bcol, 0.0,
                                          AluOpType.add, AluOpType.max)
                    # scatter into hfp rows lp = 2*lq + r (DMA: no partition
                    # alignment constraint, so scat_lo may differ from za);
                    # both s-halves of a row parity ride one DMA
                    lq0 = za if scat_lo is None else scat_lo
                    ha_ap = his_all[:]
                    hf_ap = hfp[:]
                    for r in range(2):
                        lq1 = zb
                        while 2 * (lq1 - 1) + r >= NHP:
                            lq1 -= 1
                        if lq1 <= lq0:
                            continue
                        n = lq1 - lq0
                        srcp = bass.AP(tensor=ha_ap.tensor,
                                       offset=ha_ap.offset + lq0 * 1024 + 2 * r * 256,
                                       ap=[[1024, n], [256, 2], [1, 256]])
                        dstp = bass.AP(tensor=hf_ap.tensor,
                                       offset=hf_ap.offset + (2 * lq0 + r) * 516 + 1,
                                       ap=[[2 * 516, n], [258, 2], [1, 256]])
                        eng = (nc.sync if r == 0 else nc.scalar) if tail else nc.gpsimd
                        eng.dma_start(out=dstp, in_=srcp)

                # ---- stage A: conv1 -> conv2 -> conv3 (+ DWT interleave) ----
                stA = _cl.ExitStack()
                ctx.callback(stA.close)
                xpool = stA.enter_context(tc.tile_pool(name="xpool", bufs=6))
                f1pool = stA.enter_context(tc.tile_pool(name="f1pool", bufs=5))
                f2pool = stA.enter_context(tc.tile_pool(name="f2pool", bufs=2))
                pp1 = stA.enter_context(tc.tile_pool(name="pp1", bufs=3, space="PSUM"))
                pp2 = stA.enter_context(tc.tile_pool(name="pp2", bufs=1, space="PSUM"))
                pp3 = stA.enter_context(tc.tile_pool(name="pp3", bufs=1, space="PSUM"))
                _st23 = {'ps2': None, 'ps3': None}
                _pending = []

                def _emit_conv2(g4, f1_t):
                    q = g4 % 4
                    if q == 0:
                        _st23['ps2'] = pp2.tile([128, 511], F32, tag="ps2", name=f"ps2_{g4 // 4}")
                    nc.tensor.matmul(_st23['ps2'][32 * q:32 * q + 32, :], l2, f1_t[:],
                                     start=True, stop=True, tile_position=(0, 32 * q))
                    if q == 3 or g4 == G4 - 1:
                        y16 = g4 // 4
                        f2_t = f2pool.tile([128, 511], F16, tag="f2", name=f"f2_{y16}")
                        nc.scalar.activation(f2_t[:], _st23['ps2'][:], RELU, bias=b2v, scale=1.0)
                        qq = y16 % 4
                        if qq == 0:
                            _st23['ps3'] = pp3.tile([128, 511], F32, tag="ps3",
                                                    name=f"ps3_{y16 // 4}")
                        nc.tensor.matmul(_st23['ps3'][32 * qq:32 * qq + 32, :], l3, f2_t[:],
                                         start=True, stop=True, tile_position=(0, 32 * qq))
                        t = y16 // 4
                        dstv = tmp12h[32 * qq:32 * qq + 16, t, 0:511]
                        srcv = _st23['ps3'][32 * qq:32 * qq + 16, :]
                        if y16 % 2 == 0:
                            nc.scalar.activation(
                                dstv, srcv, RELU,
                                bias=cpk32[32 * qq:32 * qq + 16, CP32_B3:CP32_B3 + 1],
                                scale=1.0)
                        else:
                            nc.vector.tensor_scalar(dstv, srcv, b3, 0.0,
                                                    AluOpType.add, AluOpType.max)

                for g4 in range(G4):
                    r0c = min(4 * g4, NT - 4)
                    xa_t = xpool.tile([128, 4, 511], F16, tag="xa", name=f"xa{g4}")
                    nc.sync.dma_start(out=xa_t[:], in_=xa_d[:, r0c:r0c + 4, :])
                    xb_t = xpool.tile([76, 4, 511], F16, tag="xb", name=f"xb{g4}")
                    nc.sync.dma_start(out=xb_t[:], in_=xb_d[:, r0c:r0c + 4, :])
                    ps1 = pp1.tile([128, 511], F32, tag="ps1", name=f"ps1_{g4}")
                    off = 4 * g4 - r0c
                    for j in range(4):
                        js = min(j + off, 3)
                        nc.tensor.matmul(ps1[32 * j:32 * j + 32, :], l1a, xa_t[:, js, :],
                                         start=True, stop=False, tile_position=(0, 32 * j))
                        nc.tensor.matmul(ps1[32 * j:32 * j + 32, :], l1b, xb_t[:, js, :],
                                         start=False, stop=True, tile_position=(0, 32 * j))
                    f1_t = f1pool.tile([128, 511], F16, tag="f1", name=f"f1_{g4}")
                    nc.scalar.activation(f1_t[:], ps1[:], RELU, bias=b1v, scale=1.0)
                    _pending.append((g4, f1_t))
                    if len(_pending) > 1:
                        _emit_conv2(*_pending.pop(0))
                    if g4 == 9:
                        emit_dwt_group(0, 16, [0, 1])
                    elif g4 == 17:
                        emit_dwt_group(0, 32, [2, 3], scat_lo=16)
                for _item in _pending:
                    _emit_conv2(*_item)
                emit_dwt_group(32, NZ, [4], tail=True)

                stA.close()
                # PE warm-keeper: the p-state model halves matmul throughput
                # after the PE idles; a stream of tiny-M matmuls over the
                # DWT/blur/stats tail keeps the clock at full speed for the
                # cat/c9a/c9b phases. Output [1,511] garbage psum.
                warm_stack = _cl.ExitStack()
                wpool = warm_stack.enter_context(
                    tc.tile_pool(name="wpool", bufs=1, space="PSUM"))
                wps = wpool.tile([1, 511], F32, name="wps")
                for _w in range(52):
                    nc.tensor.matmul(wps[:], cpk16[0:128, 0:1], cpk16[0:128, 0:511],
                                     start=True, stop=True)
                warm_stack.close()
                if dbg:
                    dt_t = mid.tile([128, 2, 512], F32, name="dt_t")
                    nc.vector.tensor_copy(dt_t[:], tmp12h[:])
                    nc.sync.dma_start(out=dtmp_d[:], in_=dt_t[:])
                if upto == 'A':
                    return

                # ---- hfp border fix (interior cols only), vertical blur ----
                kp_ap = kpk16[:]
                rbv_int = bass.AP(tensor=kp_ap.tensor,
                                  offset=kp_ap.offset + KP16_RBV + 1,
                                  ap=[[KP16_TOT, NHP], [258, 2], [1, 256]])
                nc.vector.scalar_tensor_tensor(hfp[:, :, 1:257], hfp[:, :, 1:257],
                                               rmv, rbv_int,
                                               AluOpType.mult, AluOpType.add)
                if upto == 'hfp':
                    return

                pvb_stack = _cl.ExitStack()
                pvb = pvb_stack.enter_context(tc.tile_pool(name="pvb", bufs=2, space="PSUM"))
                for s in range(2):
                    pv = pvb.tile([NH, 258], F32, tag="pv", name=f"pv{s}")
                    nc.tensor.matmul(pv[:], gv, hfp[:, s, :], start=True, stop=True)
                    nc.vector.tensor_copy(vb[:, s, :], pv[:])
                pvb_stack.close()
                g0, g1, g2, g3, g4c = [float(v) for v in g]
                ve = vb[:, 0, :]
                vo = vb[:, 1, :]

                # MAC chains accumulate in hbe/hbo scratch; the final tap
                # writes strided straight into the interleaved hb (saves the
                # separate interleave copies + one serial hop)
                def mac(eng, tmp, final_out, taps):
                    first = True
                    for (src, wv) in taps[:-1]:
                        if first:
                            eng.tensor_scalar_mul(tmp, src, float(wv))
                            first = False
                        else:
                            eng.scalar_tensor_tensor(tmp, src, float(wv), tmp,
                                                     AluOpType.mult, AluOpType.add)
                    src, wv = taps[-1]
                    eng.scalar_tensor_tensor(final_out, src, float(wv), tmp,
                                             AluOpType.mult, AluOpType.add)
                mac(nc.vector, hbe[:, 1:257], hb[0:NH, 2:514:2],
                    [(ve[:, 1:257], g2), (ve[:, 0:256], g0), (ve[:, 2:258], g4c),
                     (vo[:, 0:256], g1), (vo[:, 1:257], g3)])
                mac(nc.vector, hbo[:, 1:257], hb[0:NH, 3:515:2],
                    [(vo[:, 1:257], g2), (vo[:, 0:256], g0), (vo[:, 2:258], g4c),
                     (ve[:, 1:257], g1), (ve[:, 2:258], g3)])
                mac(nc.vector, hbe[:, 0:1], hb[0:NH, 0:1],
                    [(ve[:, 0:1], g2), (vo[:, 0:1], g1 + g3), (ve[:, 1:2], g0 + g4c)])
                mac(nc.vector, hbo[:, 0:1], hb[0:NH, 1:2],
                    [(ve[:, 0:1], g1), (vo[:, 0:1], g0 + g2), (ve[:, 1:2], g3),
                     (vo[:, 1:2], g4c)])
                mac(nc.vector, hbe[:, 257:258], hb[0:NH, 514:515],
                    [(ve[:, 256:257], g0), (vo[:, 256:257], g1),
                     (ve[:, 257:258], g2 + g4c), (vo[:, 257:258], g3)])
                mac(nc.vector, hbo[:, 257:258], hb[0:NH, 515:516],
                    [(vo[:, 256:257], g0 + g4c), (ve[:, 257:258], g1 + g3),
                     (vo[:, 257:258], g2)])
                # 1-col-shifted copy: matmul rhs needs 4B-aligned offsets, so
                # odd kx taps read hbsh at kx-1 instead of hb at kx
                nc.vector.tensor_copy(hbsh[0:NHB, 0:515], hb[0:NHB, 1:516])
                if dbg:
                    dhb_t = mid.tile([NH, 516], F32, name="dhb_t")
                    nc.vector.tensor_copy(dhb_t[:], hb[0:NH, :])
                    nc.sync.dma_start(out=dhb_d[:], in_=dhb_t[:])
                if upto == 'blur':
                    return

                # ---- stats + channel attention scalar ----
                hbm = mid.tile([NH, 516], F16, name="hbm")
                rowsum = mid.tile([NH, 2], F32, name="rowsum")
                # max path first (extra reduce op); sum fused via accum_out
                nc.vector.tensor_scalar(hbm[:], hb[0:NH, :], smv, sav,
                                        AluOpType.mult, AluOpType.add)
                nc.vector.tensor_reduce(rowsum[:, 1:2], hbm[:], mybir.AxisListType.X,
                                        AluOpType.max)
                nc.vector.tensor_scalar(hbm[:], hb[0:NH, :], smv, 0.0,
                                        AluOpType.mult, AluOpType.add,
                                        accum_out=rowsum[:, 0:1])
                # cross-partition reduction on the Pool ISA — avoids the
                # transpose-DMA hop (+~1.6us of sem latency each)
                from concourse import bass_isa
                rr2 = mid.tile([NH, 2], F32, name="rr2")
                nc.gpsimd.partition_all_reduce(rr2[:, 1:2], rowsum[:, 1:2], NH,
                                               bass_isa.ReduceOp.max)
                nc.gpsimd.partition_all_reduce(rr2[:, 0:1], rowsum[:, 0:1], NH,
                                               bass_isa.ReduceOp.add)
                nc.vector.tensor_copy(ccin[:, 0:2], rr2[0:1, 0:2])
                ccin_dr = dram.tile([1, 128], F32, name="ccin_dr")
                ccout_dr = dram.tile([8, 128], F32, name="ccout_dr", addr_space="Shared")
                nc.sync.dma_start(out=ccin_dr[:], in_=ccin[:])
                if sim:
                    nc.sync.dma_start(out=ccout_dr[:],
                                      in_=bass.AP(tensor=ccin_dr[:].tensor,
                                                  offset=ccin_dr[:].offset,
                                                  ap=[[0, NCORE], [1, 128]]))
                else:
                    nc.gpsimd.collective_compute("AllGather", AluOpType.bypass,
                                                 replica_groups=[list(range(NCORE))],
                                                 ins=[ccin_dr[:].opt()],
                                                 outs=[ccout_dr[:].opt()])
                ccsb = mid.tile([1, 8, 2], F32, name="ccsb")
                nc.sync.dma_start(out=ccsb[:], in_=ccout_dr[:, 0:2])
                gsum = mid.tile([1, 1], F32, name="gsum")
                gmax = mid.tile([1, 1], F32, name="gmax")
                nc.vector.tensor_reduce(gsum[:], ccsb[:, :, 0], mybir.AxisListType.X,
                                        AluOpType.add)
                nc.vector.tensor_reduce(gmax[:], ccsb[:, :, 1], mybir.AxisListType.X,
                                        AluOpType.max)
                ravg = mid.tile([1, 1], F32, name="ravg")
                rmx = mid.tile([1, 1], F32, name="rmx")
                nc.scalar.activation(ravg[:], gsum[:], RELU, bias=0.0,
                                     scale=float(sc['ca_w1'] / (516.0 * 516.0)))
                nc.vector.tensor_scalar(rmx[:], gmax[:], float(sc['ca_w1']), 0.0,
                                        AluOpType.mult, AluOpType.max)
                ssum = mid.tile([1, 1], F32, name="ssum")
                nc.vector.tensor_tensor(ssum[:], ravg[:], rmx[:], AluOpType.add)
                s_t = mid.tile([1, 1], F32, name="s_t")
                nc.scalar.activation(s_t[:], ssum[:], SIG, bias=0.0,
                                     scale=float(sc['ca_w2']))
                sbcast = mid.tile([128, 1], F32, name="sbcast")
                nc.gpsimd.partition_broadcast(sbcast[:], s_t[:])
                scYm = mid.tile([32, CATY], F32, name="scYm")
                nc.vector.tensor_scalar_mul(scYm[:], kpk32[0:32, KP32_FMY:KP32_FMY + CATY],
                                            sbcast[0:32, 0:1])
                if dbg:
                    dstat = mid.tile([1, 8], F32, name="dstat")
                    nc.vector.memset(dstat[:], 0.0)
                    nc.vector.tensor_copy(dstat[:, 0:1], gsum[:])
                    nc.vector.tensor_copy(dstat[:, 1:2], gmax[:])
                    nc.vector.tensor_copy(dstat[:, 2:3], s_t[:])
                    nc.vector.tensor_copy(dstat[:, 3:4], tsum[:])
                    nc.vector.tensor_copy(dstat[:, 4:5], tmax[:])
                    nc.sync.dma_start(out=dst_d[:], in_=dstat[:])
                if upto == 'B':
                    return

                # ---- cat conv (6x6 via banded K=37 matmuls, kx in PSUM) + f12 ----
                late = ctx.enter_context(tc.tile_pool(name="late", bufs=1))
                pscat = ctx.enter_context(tc.tile_pool(name="pscat", bufs=2, space="PSUM"))
                pscatB = ctx.enter_context(tc.tile_pool(name="pscatB", bufs=1, space="PSUM"))
                tmp12f = late.tile([NF, 511], F16, name="tmp12f")
                th_ap = tmp12h[:]
                for ar in range(5):
                    lo = max(16 * ar, 4)
                    hi2 = min(16 * ar + 16, 4 + NF)
                    if lo >= hi2:
                        continue
                    t = (16 * ar) // 64
                    pb = 32 * (ar % 4) + (lo - 16 * ar)
                    src = bass.AP(tensor=th_ap.tensor,
                                  offset=th_ap.offset + pb * 1024 + t * 512,
                                  ap=[[1024, hi2 - lo], [1, 511]])
                    nc.gpsimd.dma_start(out=tmp12f[lo - 4:hi2 - 4, :], in_=src)
                nc.vector.tensor_scalar_mul(tmp12f[:], tmp12f[:], tmaskc)
                if upto == 'tmpf':
                    return
                for Y in range(CATY):
                    # K=37 band split at the 32-quad boundary into two psum
                    # accumulation groups (one tile_position each; groups may
                    # not mix positions), summed at evac. Y=2's tail rows are
                    # all-zero hb padding (and base 96 is not PE-addressable),
                    # so it has no B group.
                    pscA = pscat.tile([32, 511], F32, tag="pscA", name=f"pscA{Y}")
                    pscB = pscatB.tile([32, 511], F32, tag="pscB", name=f"pscB{Y}") \
                        if Y < 2 else None
                    for kx in range(6):
                        c0 = CP16_LCAT + 192 * Y + 32 * kx
                        rhs_t, xo = (hb, kx) if kx % 2 == 0 else (hbsh, kx - 1)
                        nc.tensor.matmul(
                            pscA[:],
                            cpk16[32 * Y:32 * Y + 32, c0:c0 + 32],
                            rhs_t[32 * Y:32 * Y + 32, xo:xo + 511],
                            start=(kx == 0), stop=(kx == 5),
                            tile_position=(32 * Y, 0))
                        if pscB is not None:
                            nc.tensor.matmul(
                                pscB[:],
                                cpk16[32 * Y + 32:32 * Y + CATKR, c0:c0 + 32],
                                rhs_t[32 * Y + 32:32 * Y + CATKR, xo:xo + 511],
                                start=(kx == 0), stop=(kx == 5),
                                tile_position=(32 * Y + 32, 0))
                    if pscB is not None:
                        # ops may read only one PSUM operand: stage B via SBUF
                        catBs = late.tile([32, 511], F32, tag="catBs", name=f"catBs{Y}")
                        nc.vector.tensor_copy(catBs[:], pscB[:])
                        nc.vector.tensor_tensor(pscA[:], pscA[:], catBs[:], AluOpType.add)
                    nc.scalar.activation(f12dense[32 * Y:32 * Y + 32, 1:512], pscA[:], RELU,
                                         bias=kpk32[0:32, KP32_BCY + Y:KP32_BCY + Y + 1],
                                         scale=scYm[:, Y:Y + 1])
                    if upto == 'catmm1':
                        return
                if upto == 'catmm':
                    return
                nc.vector.tensor_tensor(f12dense[0:NF, 1:512], f12dense[0:NF, 1:512],
                                        tmp12f[:], AluOpType.add)
                if dbg:
                    df_t = late.tile([96, 513], F32, name="df_t")
                    nc.vector.tensor_copy(df_t[:], f12dense[:])
                    nc.sync.dma_start(out=df12_d[:], in_=df_t[:])
                if upto == 'cat':
                    return

                # ---- c9a (3x3, 1->32) ----
                FR = FRROWS * 513
                f12r = late.tile([18, FR], F16, name="f12r")
                fr_ap = f12r[:]
                fd_ap = f12dense[:]
                for krow in range(6):
                    for kx in range(3):
                        dst = bass.AP(tensor=fr_ap.tensor,
                                      offset=fr_ap.offset + (3 * krow + kx) * FR,
                                      ap=[[FR, 1], [513, FRROWS], [1, 511]])
                        src = bass.AP(tensor=fd_ap.tensor,
                                      offset=fd_ap.offset + krow * 513 + kx,
                                      ap=[[513, FRROWS], [1, 511]])
                        eng = [nc.sync, nc.scalar, nc.gpsimd][(3 * krow + kx) % 3]
                        eng.dma_start(out=dst, in_=src)
                ps9a_pool = ctx.enter_context(tc.tile_pool(name="ps9a", bufs=2, space="PSUM"))
                h0sb = late.tile([128, 17, 511], F16, name="h0sb")
                hreppool = ctx.enter_context(tc.tile_pool(name="hreppool", bufs=2))
                HR_FREE = 35 * 513
                hreps = {}

                def emit_hrep(hh):
                    hrep = hreppool.tile([96, HR_FREE], F16, tag="hrep", name=f"hrep{hh}")
                    hreps[hh] = hrep
                    hr_ap = hrep[:]
                    h0_ap = h0sb[:]
                    H0_FREE = 17 * 511
                    z0ap = bass.AP(tensor=hr_ap.tensor, offset=hr_ap.offset,
                                   ap=[[HR_FREE, 32], [513, 35]])
                    nc.vector.memset(z0ap, 0.0)
                    z1ap = bass.AP(tensor=hr_ap.tensor,
                                   offset=hr_ap.offset + 64 * HR_FREE + 510,
                                   ap=[[HR_FREE, 32], [513, 35]])
                    nc.vector.memset(z1ap, 0.0)
                    ndma = 0
                    for dx in range(3):
                        x0, u0, nx = (0, 1, 511) if dx == 0 else \
                            ((0, 0, 511) if dx == 1 else (1, 0, 510))
                        for j in range(4):
                            Y0 = max(0, (32 * hh - j + 3) // 4)
                            Y1 = min(17, (32 * hh + 34 - j) // 4 + 1)
                            nY = Y1 - Y0
                            if nY <= 0:
                                continue
                            rr0 = 4 * Y0 + j - 32 * hh
                            src = bass.AP(tensor=h0_ap.tensor,
                                          offset=h0_ap.offset + (32 * j) * H0_FREE
                                          + Y0 * 511 + x0,
                                          ap=[[H0_FREE, 32], [511, nY], [1, nx]])
                            dst = bass.AP(tensor=hr_ap.tensor,
                                          offset=hr_ap.offset + (32 * dx) * HR_FREE
                                          + rr0 * 513 + u0,
                                          ap=[[HR_FREE, 32], [4 * 513, nY], [1, nx]])
                            eng = [nc.sync, nc.scalar, nc.gpsimd][ndma % 3]
                            eng.dma_start(out=dst, in_=src)
                            ndma += 1

                for Y in range(17):
                    ps9a = ps9a_pool.tile([128, 511], F32, tag="ps9a", name=f"ps9a{Y}")
                    nc.tensor.matmul(ps9a[:], l9a,
                                     f12r[:, (4 * Y) * 513:(4 * Y) * 513 + 511],
                                     start=True, stop=True, tile_position=(0, 0))
                    bcol = kpk32[0:128, KP32_BAP + Y:KP32_BAP + Y + 1]
                    if Y % 2 == 0:
                        nc.scalar.activation(h0sb[:, Y, :], ps9a[:], RELU, bias=bcol,
                                             scale=1.0)
                    else:
                        nc.vector.tensor_scalar(h0sb[:, Y, :], ps9a[:], bcol, 0.0,
                                                AluOpType.add, AluOpType.max)
                    if Y == 9:
                        emit_hrep(0)
                if dbg:
                    dh_t = late.tile([128, 17, 511], F32, name="dh_t")
                    nc.vector.tensor_copy(dh_t[:], h0sb[:])
                    nc.sync.dma_start(out=dh0_d[:], in_=dh_t[:])
                if upto == 'c9a':
                    return

                # ---- c9b (3x3, 32->64) + output ----
                outpool = ctx.enter_context(tc.tile_pool(name="outpool", bufs=6))
                ps9b_pool = ctx.enter_context(tc.tile_pool(name="ps9b", bufs=3, space="PSUM"))
                o_ap = out_d[:]
                for hh in range(2):
                    hrep = hreps[hh]
                    osb = None
                    for ly2 in range(16 * hh, 16 * hh + 16):
                        if hh == 0 and ly2 == 4:
                            # emit after a few output stores so hrep1's DMAs
                            # queue behind them instead of ahead (out-store
                            # head-of-line blocking stalls the osb WAR chain)
                            emit_hrep(1)
                        rr0 = 2 * ly2 - 32 * hh
                        ps9b = ps9b_pool.tile([128, 511], F32, tag="ps9b", name=f"ps9b{ly2}")
                        for dy in range(4):
                            nc.tensor.matmul(
                                ps9b[:],
                                cpk16[0:96, CP16_L9B + 128 * dy:CP16_L9B + 128 * dy + 128],
                                hrep[:, (rr0 + dy) * 513:(rr0 + dy) * 513 + 511],
                                start=(dy == 0), stop=(dy == 3))
                        q = ly2 % 4
                        if q == 0:
                            osb = outpool.tile([128, 4, 511], F16, tag="osb",
                                               name=f"osb{ly2}")
                        if ly2 % 2 == 0:
                            nc.scalar.activation(osb[:, q, :], ps9b[:], RELU, bias=b9bv,
                                                 scale=1.0)
                        else:
                            nc.vector.tensor_scalar(osb[:, q, :], ps9b[:], b9bv, 0.0,
                                                    AluOpType.add, AluOpType.max)
                        if q == 3:
                            # out layout [p=(j2,co), ly2, x]: uniform partition
                            # stride -> one 3-dim DMA stores 4 row-pairs
                            dst = bass.AP(tensor=o_ap.tensor,
                                          offset=(ly2 - 3) * 511,
                                          ap=[[32 * 511, 128], [511, 4], [1, 511]])
                            (nc.sync if (ly2 // 4) % 2 == 0 else nc.scalar).dma_start(
                                out=dst, in_=osb[:])
        _emit()
    nc.compile()
    return nc


# ---------------- public entry point ----------------
_CACHE = {}


def kernel(**inputs):
    from concourse.bass_utils import run_bass_kernel_spmd
    inp = {k: np.asarray(v) for k, v in inputs.items()}
    sh, sc = prep_shared(inp)
    key = b"".join(np.asarray(v).tobytes() for v in
                   [sc['b3'], sc['bhi'], sc['whi'], sc['bcat'], sc['b9a'],
                    sc['ca_w1'], sc['ca_w2']])
    nc = _CACHE.get(key)
    if nc is None:
        nc = build_kernel(sc, dbg=False)
        _CACHE.clear()
        _CACHE[key] = nc
    in_maps = []
    for i in range(NCORE):
        pc = prep_core(inp, i, sc)
        in_maps.append({**sh, **pc})
    res = run_bass_kernel_spmd(nc, in_maps, core_ids=list(range(NCORE)))
    outs = []
    for i in range(NCORE):
        o = res.results[i]["out"]              # [128=(j2,co), 32=ly2, 511]
        o = o.reshape(2, 64, 32, 511).transpose(1, 2, 0, 3).reshape(64, 64, 511)
        outs.append(o[:, :64 if i < 7 else 63, :])
    full = np.concatenate(outs, axis=1)
    return full[None].astype(np.float32)
